# revision 1
# baseline (speedup 1.0000x reference)
"""DiffLogic network TRN2 kernel: 3 logic layers [B=256, W=64000] + GroupSum.

Sharding: pure data-parallel over batch across 8 cores (B=32/core), no
inter-core communication. Per core: activations h stored in DRAM as
[64000, 128] bf16 rows (256B, 32 real batch cols). Gathers a=h[idx_a],
b=h[idx_b] via SWDGE dma_gather with signed int16 indices (base at row
32000 so idx in [-32000, 32000)). Gate = c0+c1*a+c2*b+c3*ab computed on
DVE with stride-0 coefficient broadcasts; coefs = softmax(w)@G computed
on device (ACT exp + DVE reduce). GroupSum via PE one-hot matmul.
"""
import numpy as np
import ml_dtypes

import concourse.bass as bass
import concourse.tile as tile
import concourse.bacc as bacc
import concourse.mybir as mybir
from concourse.bass_utils import run_bass_kernel_spmd
from concourse.library_config import mlp
from concourse._compat import cdiv

W = 64000
BATCH = 256
NCORES = 8
BC = BATCH // NCORES        # 32 batch rows per core
IN_DIM = 1024
K = 10
TAU = 30.0
NSLOT = W // 128            # 500
E = 128                     # bf16 elements per h row (256B); [:32] real
CHUNK_SLOTS = 64            # neurons per chunk = 64*128 = 8192
GPN = 1024                  # idxs per dma_gather instruction
H_BASE = 32000              # gather base row (signed int16 rebase)

GATE_COEF = np.array([
    [0., 0., 0., 0.], [0., 0., 0., 1.], [0., 1., 0., -1.], [0., 1., 0., 0.],
    [0., 0., 1., -1.], [0., 0., 1., 0.], [0., 1., 1., -2.], [0., 1., 1., -1.],
    [1., -1., -1., 1.], [1., -1., -1., 2.], [1., 0., -1., 0.], [1., 0., -1., 1.],
    [1., -1., 0., 0.], [1., -1., 0., 1.], [1., 0., 0., -1.], [1., 0., 0., 0.],
], dtype=np.float32)  # [16, 4]

BF16 = mybir.dt.bfloat16
F32 = mybir.dt.float32
I16 = mybir.dt.int16
IDX_COLS = W // 16  # wrapped idx tensor cols per list

_NC_CACHE = {}


def _chunks():
    """Yield (slot0, nslots) chunks over the 500 slots."""
    s = 0
    while s < NSLOT:
        n = min(CHUNK_SLOTS, NSLOT - s)
        yield s, n
        s += n


def _gathers(nslots):
    """Split a chunk of nslots*128 idxs into per-instruction counts."""
    n = nslots * 128
    out = []
    while n > 0:
        g = min(GPN, n)
        out.append(g)
        n -= g
    return out


def build_nc():
    if "nc" in _NC_CACHE:
        return _NC_CACHE["nc"]
    nc = bacc.Bacc("TRN2", target_bir_lowering=False, debug=False,
                   enable_asserts=False, num_devices=NCORES)

    xT = nc.dram_tensor("xT", [IN_DIM, E], BF16, kind="ExternalInput")
    wf = [nc.dram_tensor(f"wf{l}", [128, NSLOT, 16], BF16, kind="ExternalInput")
          for l in range(3)]
    ia = [nc.dram_tensor(f"ia{l}", [128, IDX_COLS], I16, kind="ExternalInput")
          for l in range(3)]
    ib = [nc.dram_tensor(f"ib{l}", [128, IDX_COLS], I16, kind="ExternalInput")
          for l in range(3)]
    g10 = nc.dram_tensor("g10", [128, NSLOT, K], BF16, kind="ExternalInput")
    gmat = nc.dram_tensor("gmat", [128, 5, 16], BF16, kind="ExternalInput")
    h_dram = [nc.dram_tensor(f"h{l}", [W, E], BF16, kind="Internal")
              for l in range(2)]
    out_dram = nc.dram_tensor("out", [K, BC], F32, kind="ExternalOutput")

    with tile.TileContext(nc) as tc:
        with (
            tc.tile_pool(name="persist", bufs=1) as persist,
            tc.tile_pool(name="coef", bufs=1) as coefp,
            tc.tile_pool(name="gath", bufs=2) as gath,
            tc.tile_pool(name="temps", bufs=2) as temps,
            tc.tile_pool(name="psum", bufs=1, space="PSUM") as psump,
        ):
            nc.gpsimd.load_library(mlp)

            # persistent loads
            gmat_sb = persist.tile([128, 5, 16], BF16, tag="gmat")
            nc.sync.dma_start(gmat_sb[:], gmat[:])
            g10_sb = persist.tile([128, NSLOT, K], BF16, tag="g10")
            nc.sync.dma_start(g10_sb[:], g10[:])

            psum_out = psump.tile([K, BC], F32, tag="acc")
            n_mm = NSLOT  # total groupsum matmuls
            mm_i = 0

            for l in range(3):
                ia_sb = persist.tile([128, IDX_COLS], I16, tag="ia", name="ia_sb")
                ib_sb = persist.tile([128, IDX_COLS], I16, tag="ib", name="ib_sb")
                nc.sync.dma_start(ia_sb[:], ia[l][:])
                nc.sync.dma_start(ib_sb[:], ib[l][:])

                # ---- coefficient prep: coef = softmax(wf) @ GATE_COEF ----
                HS = NSLOT // 2
                cj = [coefp.tile([128, NSLOT], BF16, tag=f"c{j}", name=f"cj{j}") for j in range(4)]
                for h in range(2):
                    hs = slice(h * HS, (h + 1) * HS)
                    wf_sb = coefp.tile([128, HS, 16], BF16, tag="wf", name="wf_sb")
                    nc.sync.dma_start(wf_sb[:], wf[l][:, hs, :])
                    e_sb = coefp.tile([128, HS, 16], BF16, tag="e", name="e_sb")
                    nc.scalar.activation(e_sb[:], wf_sb[:],
                                         mybir.ActivationFunctionType.Exp)
                    prod = coefp.tile([128, HS, 16], BF16, tag="prod", name="prod")
                    craw = [coefp.tile([128, HS], F32, tag=f"craw{j}", name=f"craw{j}")
                            for j in range(4)]
                    for j in range(4):
                        gj = gmat_sb[:, j, :].unsqueeze(1).to_broadcast([128, HS, 16])
                        nc.vector.tensor_mul(prod[:], e_sb[:], gj)
                        nc.vector.tensor_reduce(craw[j][:], prod[:],
                                                mybir.AxisListType.X,
                                                mybir.AluOpType.add)
                    ssum = coefp.tile([128, HS], F32, tag="ssum", name="ssum")
                    nc.vector.tensor_reduce(ssum[:], e_sb[:], mybir.AxisListType.X,
                                            mybir.AluOpType.add)
                    rinv = coefp.tile([128, HS], F32, tag="rinv", name="rinv")
                    nc.vector.reciprocal(out=rinv[:], in_=ssum[:])
                    for j in range(4):
                        nc.vector.tensor_mul(cj[j][:, hs], craw[j][:], rinv[:])

                # ---- gather + gate over chunks ----
                if l == 0:
                    src_ap = xT[:]
                else:
                    src_ap = h_dram[l - 1][H_BASE:W]

                for s0, ns in _chunks():
                    a_t = gath.tile([128, CHUNK_SLOTS, E], BF16, tag="a")
                    b_t = gath.tile([128, CHUNK_SLOTS, E], BF16, tag="b")
                    col = s0 * 8  # idx cols consumed so far (128/16 per slot)
                    slot = 0
                    for n in _gathers(ns):
                        ncols = n // 16
                        nslots_g = n // 128
                        nc.gpsimd.dma_gather(
                            a_t[:, slot:slot + nslots_g, :], src_ap,
                            ia_sb[:, col:col + ncols], n, n, E)
                        nc.gpsimd.dma_gather(
                            b_t[:, slot:slot + nslots_g, :], src_ap,
                            ib_sb[:, col:col + ncols], n, n, E)
                        col += ncols
                        slot += nslots_g

                    av = a_t[:, :ns, :32]
                    bv = b_t[:, :ns, :32]

                    def cbc(j):
                        return (cj[j][:, s0:s0 + ns].unsqueeze(-1)
                                .to_broadcast([128, ns, 32]))

                    t_t = temps.tile([128, CHUNK_SLOTS, 32], BF16, tag="t")
                    u_t = temps.tile([128, CHUNK_SLOTS, 32], BF16, tag="u")
                    v_t = temps.tile([128, CHUNK_SLOTS, 32], BF16, tag="v")
                    w_t = temps.tile([128, CHUNK_SLOTS, 32], BF16, tag="w")
                    nc.vector.tensor_mul(t_t[:, :ns, :], av, bv)
                    nc.vector.tensor_mul(u_t[:, :ns, :], t_t[:, :ns, :], cbc(3))
                    nc.vector.tensor_mul(v_t[:, :ns, :], av, cbc(1))
                    nc.vector.tensor_mul(w_t[:, :ns, :], bv, cbc(2))
                    nc.vector.tensor_add(u_t[:, :ns, :], u_t[:, :ns, :], v_t[:, :ns, :])
                    nc.vector.tensor_add(w_t[:, :ns, :], w_t[:, :ns, :], cbc(0))
                    nc.vector.tensor_add(t_t[:, :ns, :], u_t[:, :ns, :], w_t[:, :ns, :])

                    if l < 2:
                        # write rows (s0+c)*128+p of h_dram[l]
                        hap = h_dram[l].ap()
                        dst = hap[s0 * 128: s0 * 128 + ns * 128, :32]
                        dst = dst.rearrange("(c p) e -> p c e", p=128)
                        nc.gpsimd.dma_start(dst, t_t[:, :ns, :])
                    else:
                        for c in range(ns):
                            nc.tensor.matmul(
                                psum_out[:],
                                lhsT=g10_sb[:, s0 + c, :],
                                rhs=t_t[:, c, :],
                                start=(mm_i == 0),
                                stop=(mm_i == n_mm - 1),
                            )
                            mm_i += 1

            out_sb = persist.tile([K, BC], F32, tag="outsb")
            nc.scalar.mul(out_sb[:], psum_out[:], 1.0 / TAU)
            nc.sync.dma_start(out_dram[:], out_sb[:])

    nc.compile()
    _NC_CACHE["nc"] = nc
    return nc


def _wrap(idx):
    """Flat idx list [n] -> [128, n/16] int16 wrapped per 16 partitions,
    replicated to the 8 gpsimd cores."""
    n = idx.shape[0]
    arr = np.empty((128, n // 16), dtype=np.int16)
    blk = idx.reshape(n // 16, 16).T.astype(np.int16)
    for g in range(8):
        arr[g * 16:(g + 1) * 16, :] = blk
    return arr


def _fix_trailing(idx_a, idx_b):
    """Ensure the last idx of every GPN-sublist is >= 0 for both lists
    (SWDGE trims trailing negatives). Returns permuted lists + perm."""
    perm = np.arange(W)
    a = idx_a.copy()
    b = idx_b.copy()
    pos = 0
    for s0, ns in _chunks():
        for n in _gathers(ns):
            last = pos + n - 1
            if a[last] < 0 or b[last] < 0:
                ok = np.nonzero((a[pos:last] >= 0) & (b[pos:last] >= 0))[0]
                j = pos + int(ok[-1])
                for arr in (a, b, perm):
                    arr[last], arr[j] = arr[j], arr[last]
            pos += n
    return a, b, perm


def _fold(x):
    """[W, ...] -> [128, NSLOT, ...] with row n=(c*128+p) at [p, c]."""
    return np.ascontiguousarray(
        x.reshape(NSLOT, 128, *x.shape[1:]).transpose(1, 0, *range(2, x.ndim + 1)))


def kernel(x, w1, w2, w3, idx_a1, idx_b1, idx_a2, idx_b2, idx_a3, idx_b3):
    x = np.asarray(x, dtype=np.float32)
    ws = [np.asarray(w, dtype=np.float32) for w in (w1, w2, w3)]
    ias = [np.asarray(i).astype(np.int64) for i in (idx_a1, idx_a2, idx_a3)]
    ibs = [np.asarray(i).astype(np.int64) for i in (idx_b1, idx_b2, idx_b3)]

    nc = build_nc()

    # ---- host-side index translation / layout prep (shared across cores) ----
    # layer 0: sources are x columns (0..1023), no rebase needed
    a0, b0, perm0 = ias[0].copy(), ibs[0].copy(), np.arange(W)
    perms = [perm0]
    lists = [(a0, b0)]
    for l in (1, 2):
        inv_prev = np.empty(W, dtype=np.int64)
        inv_prev[perms[l - 1]] = np.arange(W)
        ra = inv_prev[ias[l]] - H_BASE
        rb = inv_prev[ibs[l]] - H_BASE
        ra2, rb2, perm = _fix_trailing(ra, rb)
        perms.append(perm)
        lists.append((ra2, rb2))

    shared = {}
    for l in range(3):
        a, b = lists[l]
        shared[f"ia{l}"] = _wrap(a)
        shared[f"ib{l}"] = _wrap(b)
        shared[f"wf{l}"] = _fold(ws[l][perms[l]]).astype(ml_dtypes.bfloat16)

    group = perms[2] // (W // K)          # group id of neuron at list pos j
    g10 = np.zeros((W, K), dtype=np.float32)
    g10[np.arange(W), group] = 1.0
    shared["g10"] = _fold(g10).astype(ml_dtypes.bfloat16)

    gm = np.zeros((5, 16), dtype=np.float32)
    gm[:4] = GATE_COEF.T
    gm[4] = 1.0
    shared["gmat"] = np.broadcast_to(gm, (128, 5, 16)).astype(ml_dtypes.bfloat16)

    in_maps = []
    for c in range(NCORES):
        xc = x[c * BC:(c + 1) * BC]               # [32, 1024]
        xt = np.zeros((IN_DIM, E), dtype=ml_dtypes.bfloat16)
        xt[:, :BC] = xc.T.astype(ml_dtypes.bfloat16)
        m = dict(shared)
        m["xT"] = xt
        in_maps.append(m)

    res = run_bass_kernel_spmd(nc, in_maps, core_ids=list(range(NCORES)))

    out = np.empty((BATCH, K), dtype=np.float32)
    for c in range(NCORES):
        out[c * BC:(c + 1) * BC] = res.results[c]["out"].T
    return out



# revision 2
# speedup vs baseline: 1.0838x; 1.0838x over previous
"""DiffLogic network TRN2 kernel: 3 logic layers [B=256, W=64000] + GroupSum.

Sharding: pure data-parallel over batch across 8 cores (B=32/core), no
inter-core communication. Per core: activations h stored in DRAM as
[64000, 128] bf16 rows (256B stride, first 32 cols real = 64B payload).
Gathers a=h[idx_a], b=h[idx_b] via SWDGE dma_gather with elem_size=32
(64B descriptors — 7ns floor instead of 22.8ns for 256B) and signed
int16 indices (base at row 32000). h writes go over HWDGE (SP engine)
to keep the Pool engine free for gather descriptor generation.
Gate = (c1+c3*b)*a + (c0+c2*b) computed on DVE (6 tensor-tensor ops);
coefs = softmax(w)@G on ACT (exp) + DVE (reduce). GroupSum via PE
one-hot matmul.
"""
import numpy as np
import ml_dtypes

import concourse.bass as bass
import concourse.tile as tile
import concourse.bacc as bacc
import concourse.mybir as mybir
from concourse import ap_utils
from concourse.bass_utils import run_bass_kernel_spmd
from concourse.library_config import mlp

W = 64000
BATCH = 256
NCORES = 8
BC = BATCH // NCORES        # 32 batch rows per core
IN_DIM = 1024
K = 10
TAU = 30.0
NSLOT = W // 128            # 500
E = 128                     # elements per h row (256B stride); [:32] real
CHUNK_SLOTS = 64            # neurons per chunk = 64*128 = 8192
GPN = 1024                  # idxs per dma_gather instruction (Q7 scratch cap)
H_BASE = 32000              # gather base row (signed int16 rebase)

GATE_COEF = np.array([
    [0., 0., 0., 0.], [0., 0., 0., 1.], [0., 1., 0., -1.], [0., 1., 0., 0.],
    [0., 0., 1., -1.], [0., 0., 1., 0.], [0., 1., 1., -2.], [0., 1., 1., -1.],
    [1., -1., -1., 1.], [1., -1., -1., 2.], [1., 0., -1., 0.], [1., 0., -1., 1.],
    [1., -1., 0., 0.], [1., -1., 0., 1.], [1., 0., 0., -1.], [1., 0., 0., 0.],
], dtype=np.float32)  # [16, 4]

BF16 = mybir.dt.bfloat16
F32 = mybir.dt.float32
I16 = mybir.dt.int16
IDX_COLS = W // 16  # wrapped idx tensor cols per list

_NC_CACHE = {}


def dma_gather_small(gp, out_ap, in_ap, idxs_ap, num_idxs, elem_size, elem_step):
    """dma_gather without the 256B elem-size assert (non-transpose, DRAM src).
    The 256B restriction only applies to transpose mode in the ucode; the
    source row stride (elem_step) must still be a multiple of 256B."""
    assert idxs_ap.dtype == mybir.dt.int16
    assert in_ap.dtype == out_ap.dtype
    assert ap_utils.ap_is_contiguous(out_ap.ap[1:])
    assert ap_utils.ap_is_contiguous(idxs_ap.ap[1:])
    assert in_ap.ap[-1][1] == out_ap.ap[-1][1] == elem_size
    assert in_ap.ap[0][0] == elem_step
    stride_bytes = elem_step * mybir.dt.size(in_ap.dtype)
    stride_256 = stride_bytes // 256
    assert stride_256 * 256 == stride_bytes and stride_256 < 256
    _in_ap = gp.lower_ap_dma(in_ap, for_custom_bir_dma=True)
    _idxs_ap = gp.lower_ap(idxs_ap)
    _out_ap = gp.lower_ap(out_ap)
    return gp.add_instruction(
        mybir.InstDMAGatherAnt(
            name=gp.bass.get_next_instruction_name(),
            ins=[*_in_ap, _idxs_ap, gp.lower_val_access(gp.to_reg(num_idxs))],
            outs=[_out_ap],
            transpose=False,
            num_idxs=num_idxs,
            elem_size=elem_size,
            stride_bytes_256=stride_256,
            gen_mode=0,
            single_packet=True,
            queue_num=0,
            sbuf_tokens_per_rank=0,
            sbuf_free_dim_per_rank=0,
            sbuf_free_dim_pad_per_rank=0,
            sbuf_byte_offset=0,
        ))


def _chunks():
    """Yield (slot0, nslots) chunks over the 500 slots."""
    s = 0
    while s < NSLOT:
        n = min(CHUNK_SLOTS, NSLOT - s)
        yield s, n
        s += n


def _gathers(nslots):
    """Split a chunk of nslots*128 idxs into per-instruction counts."""
    n = nslots * 128
    out = []
    while n > 0:
        g = min(GPN, n)
        out.append(g)
        n -= g
    return out


def build_nc():
    if "nc" in _NC_CACHE:
        return _NC_CACHE["nc"]
    nc = bacc.Bacc("TRN2", target_bir_lowering=False, debug=False,
                   enable_asserts=False, num_devices=NCORES)

    xT = nc.dram_tensor("xT", [IN_DIM, E], BF16, kind="ExternalInput")
    wf = [nc.dram_tensor(f"wf{l}", [128, NSLOT, 16], BF16, kind="ExternalInput")
          for l in range(3)]
    ia = [nc.dram_tensor(f"ia{l}", [128, IDX_COLS], I16, kind="ExternalInput")
          for l in range(3)]
    ib = [nc.dram_tensor(f"ib{l}", [128, IDX_COLS], I16, kind="ExternalInput")
          for l in range(3)]
    g10 = nc.dram_tensor("g10", [128, NSLOT, K], BF16, kind="ExternalInput")
    gmat = nc.dram_tensor("gmat", [128, 5, 16], BF16, kind="ExternalInput")
    h_dram = [nc.dram_tensor(f"h{l}", [W, E], BF16, kind="Internal")
              for l in range(2)]
    out_dram = nc.dram_tensor("out", [K, BC], F32, kind="ExternalOutput")

    with tile.TileContext(nc) as tc:
        with (
            tc.tile_pool(name="persist", bufs=1) as persist,
            tc.tile_pool(name="coef", bufs=1) as coefp,
            tc.tile_pool(name="gath", bufs=2) as gath,
            tc.tile_pool(name="temps", bufs=2) as temps,
            tc.tile_pool(name="psum", bufs=1, space="PSUM") as psump,
        ):
            nc.gpsimd.load_library(mlp)

            # persistent loads
            gmat_sb = persist.tile([128, 5, 16], BF16, tag="gmat")
            nc.sync.dma_start(gmat_sb[:], gmat[:])
            g10_sb = persist.tile([128, NSLOT, K], BF16, tag="g10")
            nc.sync.dma_start(g10_sb[:], g10[:])

            psum_out = psump.tile([K, BC], F32, tag="acc")
            n_mm = NSLOT  # total groupsum matmuls
            mm_i = 0

            for l in range(3):
                ia_sb = persist.tile([128, IDX_COLS], I16, tag="ia", name="ia_sb")
                ib_sb = persist.tile([128, IDX_COLS], I16, tag="ib", name="ib_sb")
                nc.sync.dma_start(ia_sb[:], ia[l][:])
                nc.sync.dma_start(ib_sb[:], ib[l][:])

                # ---- coefficient prep: coef = softmax(wf) @ GATE_COEF ----
                HS = NSLOT // 2
                cj = [coefp.tile([128, NSLOT], BF16, tag=f"c{j}", name=f"cj{j}") for j in range(4)]
                for h in range(2):
                    hs = slice(h * HS, (h + 1) * HS)
                    wf_sb = coefp.tile([128, HS, 16], BF16, tag="wf", name="wf_sb")
                    nc.sync.dma_start(wf_sb[:], wf[l][:, hs, :])
                    e_sb = coefp.tile([128, HS, 16], BF16, tag="e", name="e_sb")
                    nc.scalar.activation(e_sb[:], wf_sb[:],
                                         mybir.ActivationFunctionType.Exp)
                    prod = coefp.tile([128, HS, 16], BF16, tag="prod", name="prod")
                    craw = [coefp.tile([128, HS], F32, tag=f"craw{j}", name=f"craw{j}")
                            for j in range(4)]
                    for j in range(4):
                        gj = gmat_sb[:, j, :].unsqueeze(1).to_broadcast([128, HS, 16])
                        nc.vector.tensor_mul(prod[:], e_sb[:], gj)
                        nc.vector.tensor_reduce(craw[j][:], prod[:],
                                                mybir.AxisListType.X,
                                                mybir.AluOpType.add)
                    ssum = coefp.tile([128, HS], F32, tag="ssum", name="ssum")
                    nc.vector.tensor_reduce(ssum[:], e_sb[:], mybir.AxisListType.X,
                                            mybir.AluOpType.add)
                    rinv = coefp.tile([128, HS], F32, tag="rinv", name="rinv")
                    nc.vector.reciprocal(out=rinv[:], in_=ssum[:])
                    for j in range(4):
                        nc.vector.tensor_mul(cj[j][:, hs], craw[j][:], rinv[:])

                # ---- gather + gate over chunks ----
                if l == 0:
                    src_ap = xT[:, :32]
                else:
                    src_ap = h_dram[l - 1][H_BASE:W, :32]

                for s0, ns in _chunks():
                    a_t = gath.tile([128, CHUNK_SLOTS, 32], BF16, tag="a")
                    b_t = gath.tile([128, CHUNK_SLOTS, 32], BF16, tag="b")
                    col = s0 * 8  # idx cols consumed so far (128/16 per slot)
                    slot = 0
                    for n in _gathers(ns):
                        ncols = n // 16
                        nslots_g = n // 128
                        dma_gather_small(
                            nc.gpsimd, a_t[:, slot:slot + nslots_g, :], src_ap,
                            ia_sb[:, col:col + ncols], n, 32, E)
                        dma_gather_small(
                            nc.gpsimd, b_t[:, slot:slot + nslots_g, :], src_ap,
                            ib_sb[:, col:col + ncols], n, 32, E)
                        col += ncols
                        slot += nslots_g

                    av = a_t[:, :ns, :]
                    bv = b_t[:, :ns, :]

                    def cbc(j):
                        return (cj[j][:, s0:s0 + ns].unsqueeze(-1)
                                .to_broadcast([128, ns, 32]))

                    # gate = (c1 + c3*b)*a + (c0 + c2*b): 6 DVE ops
                    m1 = temps.tile([128, CHUNK_SLOTS, 32], BF16, tag="m1")
                    m2 = temps.tile([128, CHUNK_SLOTS, 32], BF16, tag="m2")
                    nc.vector.tensor_mul(m1[:, :ns, :], bv, cbc(3))
                    nc.vector.tensor_add(m1[:, :ns, :], m1[:, :ns, :], cbc(1))
                    nc.vector.tensor_mul(m1[:, :ns, :], m1[:, :ns, :], av)
                    nc.vector.tensor_mul(m2[:, :ns, :], bv, cbc(2))
                    nc.vector.tensor_add(m2[:, :ns, :], m2[:, :ns, :], cbc(0))
                    nc.vector.tensor_add(m1[:, :ns, :], m1[:, :ns, :], m2[:, :ns, :])

                    if l < 2:
                        # write rows (s0+c)*128+p of h_dram[l] (64B @ 256B stride)
                        hap = h_dram[l].ap()
                        dst = hap[s0 * 128: s0 * 128 + ns * 128, :32]
                        dst = dst.rearrange("(c p) e -> p c e", p=128)
                        nc.sync.dma_start(dst, m1[:, :ns, :])
                    else:
                        for c in range(ns):
                            nc.tensor.matmul(
                                psum_out[:],
                                lhsT=g10_sb[:, s0 + c, :],
                                rhs=m1[:, c, :],
                                start=(mm_i == 0),
                                stop=(mm_i == n_mm - 1),
                            )
                            mm_i += 1

            out_sb = persist.tile([K, BC], F32, tag="outsb")
            nc.scalar.mul(out_sb[:], psum_out[:], 1.0 / TAU)
            nc.sync.dma_start(out_dram[:], out_sb[:])

    nc.compile()
    _NC_CACHE["nc"] = nc
    return nc


def _wrap(idx):
    """Flat idx list [n] -> [128, n/16] int16 wrapped per 16 partitions,
    replicated to the 8 gpsimd cores."""
    n = idx.shape[0]
    arr = np.empty((128, n // 16), dtype=np.int16)
    blk = idx.reshape(n // 16, 16).T.astype(np.int16)
    for g in range(8):
        arr[g * 16:(g + 1) * 16, :] = blk
    return arr


def _fix_trailing(idx_a, idx_b):
    """Ensure the last idx of every GPN-sublist is >= 0 for both lists
    (SWDGE trims trailing negatives). Returns permuted lists + perm."""
    perm = np.arange(W)
    a = idx_a.copy()
    b = idx_b.copy()
    pos = 0
    for s0, ns in _chunks():
        for n in _gathers(ns):
            last = pos + n - 1
            if a[last] < 0 or b[last] < 0:
                ok = np.nonzero((a[pos:last] >= 0) & (b[pos:last] >= 0))[0]
                j = pos + int(ok[-1])
                for arr in (a, b, perm):
                    arr[last], arr[j] = arr[j], arr[last]
            pos += n
    return a, b, perm


def _fold(x):
    """[W, ...] -> [128, NSLOT, ...] with row n=(c*128+p) at [p, c]."""
    return np.ascontiguousarray(
        x.reshape(NSLOT, 128, *x.shape[1:]).transpose(1, 0, *range(2, x.ndim + 1)))


def kernel(x, w1, w2, w3, idx_a1, idx_b1, idx_a2, idx_b2, idx_a3, idx_b3):
    x = np.asarray(x, dtype=np.float32)
    ws = [np.asarray(w, dtype=np.float32) for w in (w1, w2, w3)]
    ias = [np.asarray(i).astype(np.int64) for i in (idx_a1, idx_a2, idx_a3)]
    ibs = [np.asarray(i).astype(np.int64) for i in (idx_b1, idx_b2, idx_b3)]

    nc = build_nc()

    # ---- host-side index translation / layout prep (shared across cores) ----
    # layer 0: sources are x columns (0..1023), no rebase needed
    a0, b0, perm0 = ias[0].copy(), ibs[0].copy(), np.arange(W)
    perms = [perm0]
    lists = [(a0, b0)]
    for l in (1, 2):
        inv_prev = np.empty(W, dtype=np.int64)
        inv_prev[perms[l - 1]] = np.arange(W)
        ra = inv_prev[ias[l]] - H_BASE
        rb = inv_prev[ibs[l]] - H_BASE
        ra2, rb2, perm = _fix_trailing(ra, rb)
        perms.append(perm)
        lists.append((ra2, rb2))

    shared = {}
    for l in range(3):
        a, b = lists[l]
        shared[f"ia{l}"] = _wrap(a)
        shared[f"ib{l}"] = _wrap(b)
        shared[f"wf{l}"] = _fold(ws[l][perms[l]]).astype(ml_dtypes.bfloat16)

    group = perms[2] // (W // K)          # group id of neuron at list pos j
    g10 = np.zeros((W, K), dtype=np.float32)
    g10[np.arange(W), group] = 1.0
    shared["g10"] = _fold(g10).astype(ml_dtypes.bfloat16)

    gm = np.zeros((5, 16), dtype=np.float32)
    gm[:4] = GATE_COEF.T
    gm[4] = 1.0
    shared["gmat"] = np.broadcast_to(gm, (128, 5, 16)).astype(ml_dtypes.bfloat16)

    in_maps = []
    for c in range(NCORES):
        xc = x[c * BC:(c + 1) * BC]               # [32, 1024]
        xt = np.zeros((IN_DIM, E), dtype=ml_dtypes.bfloat16)
        xt[:, :BC] = xc.T.astype(ml_dtypes.bfloat16)
        m = dict(shared)
        m["xT"] = xt
        in_maps.append(m)

    res = run_bass_kernel_spmd(nc, in_maps, core_ids=list(range(NCORES)))

    out = np.empty((BATCH, K), dtype=np.float32)
    for c in range(NCORES):
        out[c * BC:(c + 1) * BC] = res.results[c]["out"].T
    return out


# revision 8
# speedup vs baseline: 1.2438x; 1.1476x over previous
"""DiffLogic network TRN2 kernel: 3 logic layers [B=256, W=64000] + GroupSum.

Sharding: pure data-parallel over batch across 8 cores (B=32/core), no
inter-core communication. Per core: activations h stored in DRAM as
[64000, 128] bf16 rows (256B stride, first 32 cols real = 64B payload).
Gathers a=h[idx_a], b=h[idx_b] via SWDGE dma_gather with elem_size=32
(64B descriptors — 7ns floor instead of 22.8ns for 256B) and signed
int16 indices (base at row 32000). h writes go over HWDGE (SP engine)
to keep the Pool engine free for gather descriptor generation.
Gate = (c1+c3*b)*a + (c0+c2*b) computed on DVE (6 tensor-tensor ops);
coefs = softmax(w)@G on ACT (exp) + DVE (reduce). GroupSum via PE
one-hot matmul.
"""
import numpy as np
import ml_dtypes

import concourse.bass as bass
import concourse.tile as tile
import concourse.bacc as bacc
import concourse.mybir as mybir
from concourse import ap_utils
from concourse.bass_utils import run_bass_kernel_spmd
from concourse.library_config import mlp

W = 64000
BATCH = 256
NCORES = 8
BC = BATCH // NCORES        # 32 batch rows per core
IN_DIM = 1024
K = 10
TAU = 30.0
NSLOT = W // 128            # 500
E = 128                     # elements per h row (256B stride); [:32] real
CHUNK_SLOTS = 64            # neurons per chunk = 64*128 = 8192
GPN = 1024                  # idxs per dma_gather instruction (Q7 scratch cap)
H_BASE = 32000              # gather base row (signed int16 rebase)

GATE_COEF = np.array([
    [0., 0., 0., 0.], [0., 0., 0., 1.], [0., 1., 0., -1.], [0., 1., 0., 0.],
    [0., 0., 1., -1.], [0., 0., 1., 0.], [0., 1., 1., -2.], [0., 1., 1., -1.],
    [1., -1., -1., 1.], [1., -1., -1., 2.], [1., 0., -1., 0.], [1., 0., -1., 1.],
    [1., -1., 0., 0.], [1., -1., 0., 1.], [1., 0., 0., -1.], [1., 0., 0., 0.],
], dtype=np.float32)  # [16, 4]

BF16 = mybir.dt.bfloat16
F32 = mybir.dt.float32
I16 = mybir.dt.int16
IDX_COLS = W // 16  # wrapped idx tensor cols per list

_NC_CACHE = {}


def dma_gather_small(gp, out_ap, in_ap, idxs_ap, num_idxs, elem_size, elem_step):
    """dma_gather without the 256B elem-size assert (non-transpose, DRAM src).
    The 256B restriction only applies to transpose mode in the ucode; the
    source row stride (elem_step) must still be a multiple of 256B."""
    assert idxs_ap.dtype == mybir.dt.int16
    assert in_ap.dtype == out_ap.dtype
    assert ap_utils.ap_is_contiguous(out_ap.ap[1:])
    assert ap_utils.ap_is_contiguous(idxs_ap.ap[1:])
    assert in_ap.ap[-1][1] == out_ap.ap[-1][1] == elem_size
    assert in_ap.ap[0][0] == elem_step
    stride_bytes = elem_step * mybir.dt.size(in_ap.dtype)
    stride_256 = stride_bytes // 256
    assert stride_256 * 256 == stride_bytes and stride_256 < 256
    _in_ap = gp.lower_ap_dma(in_ap, for_custom_bir_dma=True)
    _idxs_ap = gp.lower_ap(idxs_ap)
    _out_ap = gp.lower_ap(out_ap)
    return gp.add_instruction(
        mybir.InstDMAGatherAnt(
            name=gp.bass.get_next_instruction_name(),
            ins=[*_in_ap, _idxs_ap, gp.lower_val_access(gp.to_reg(num_idxs))],
            outs=[_out_ap],
            transpose=False,
            num_idxs=num_idxs,
            elem_size=elem_size,
            stride_bytes_256=stride_256,
            gen_mode=0,
            single_packet=True,
            queue_num=0,
            sbuf_tokens_per_rank=0,
            sbuf_free_dim_per_rank=0,
            sbuf_free_dim_pad_per_rank=0,
            sbuf_byte_offset=0,
        ))


def _chunks():
    """Yield (slot0, nslots) chunks over the 500 slots."""
    s = 0
    while s < NSLOT:
        n = min(CHUNK_SLOTS, NSLOT - s)
        yield s, n
        s += n


def _gathers(nslots):
    """Split a chunk of nslots*128 idxs into per-instruction counts."""
    n = nslots * 128
    out = []
    while n > 0:
        g = min(GPN, n)
        out.append(g)
        n -= g
    return out


def build_nc():
    if "nc" in _NC_CACHE:
        return _NC_CACHE["nc"]
    nc = bacc.Bacc("TRN2", target_bir_lowering=False, debug=False,
                   enable_asserts=False, num_devices=NCORES)

    NQ = (NSLOT + 7) // 8  # 63 PE-coef matmuls (8 slots each, 4 pad)

    xT = nc.dram_tensor("xT", [IN_DIM, E], BF16, kind="ExternalInput")
    # wf2[(k,m), q, K] = w[neuron(p=K, s=8q+m), k]  (PE-coef layout)
    wf = [nc.dram_tensor(f"wf{l}", [128, NQ, 128], BF16, kind="ExternalInput")
          for l in range(3)]
    ia = [nc.dram_tensor(f"ia{l}", [128, IDX_COLS], I16, kind="ExternalInput")
          for l in range(3)]
    ib = [nc.dram_tensor(f"ib{l}", [128, IDX_COLS], I16, kind="ExternalInput")
          for l in range(3)]
    g10 = nc.dram_tensor("g10", [128, NSLOT, K], BF16, kind="ExternalInput")
    # ghat[(k,m), 5j+mm] = G[k, j] * (m == mm); j=4 row is ones (softmax denom)
    ghat = nc.dram_tensor("ghat", [128, 40], BF16, kind="ExternalInput")
    h_dram = [nc.dram_tensor(f"h{l}", [W, E], BF16, kind="Internal")
              for l in range(2)]
    out_dram = nc.dram_tensor("out", [K, BC], F32, kind="ExternalOutput")

    with tile.TileContext(nc) as tc:
        with (
            tc.tile_pool(name="persist", bufs=1) as persist,
            tc.tile_pool(name="coef", bufs=1) as coefp,
            tc.tile_pool(name="gath", bufs=2) as gath,
            tc.tile_pool(name="temps", bufs=2) as temps,
            tc.tile_pool(name="psum", bufs=1, space="PSUM") as psump,
        ):
            nc.gpsimd.load_library(mlp)

            # persistent loads
            ghat_sb = persist.tile([128, 40], BF16, tag="ghat")
            nc.sync.dma_start(ghat_sb[:], ghat[:])
            g10_sb = persist.tile([128, NSLOT, K], BF16, tag="g10")
            nc.sync.dma_start(g10_sb[:], g10[:])

            psum_out = psump.tile([K, BC], F32, tag="acc")
            n_mm = NSLOT  # total groupsum matmuls
            mm_i = 0

            for l in range(3):
                ia_sb = persist.tile([128, IDX_COLS], I16, tag="ia", name="ia_sb")
                ib_sb = persist.tile([128, IDX_COLS], I16, tag="ib", name="ib_sb")
                nc.sync.dma_start(ia_sb[:], ia[l][:])
                nc.sync.dma_start(ib_sb[:], ib[l][:])

                # ---- coefficient prep: coef = softmax(w) @ GATE_COEF via PE ----
                # craw[j, n] for neurons n=(K, s=8q+m): matmul q contracts over
                # the (gate k, m) partition dim: out[K, j*8+mm] = sum_p
                # e_fold[p, K] * ghat[p, j*8+mm].
                wf_sb = coefp.tile([128, NQ, 128], BF16, tag="wf", name="wf_sb")
                nc.sync.dma_start(wf_sb[:], wf[l][:])
                e_sb = coefp.tile([128, NQ, 128], BF16, tag="e", name="e_sb")
                nc.scalar.activation(e_sb[:], wf_sb[:],
                                     mybir.ActivationFunctionType.Exp)
                c_sb = coefp.tile([128, NQ, 40], BF16, tag="csb", name="c_sb")
                QG = 12  # matmuls per PSUM bank group
                q0 = 0
                while q0 < NQ:
                    nq = min(QG, NQ - q0)
                    cps = psump.tile([128, QG, 40], F32, tag="cps", name="cps")
                    for qi in range(nq):
                        nc.tensor.matmul(cps[:, qi, :],
                                         lhsT=e_sb[:, q0 + qi, :],
                                         rhs=ghat_sb[:],
                                         start=True, stop=True)
                    nc.scalar.mul(c_sb[:, q0:q0 + nq, :], cps[:, :nq, :], 1.0)
                    q0 += nq
                rinv = coefp.tile([128, NQ, 8], F32, tag="rinv", name="rinv")
                nc.vector.reciprocal(out=rinv[:], in_=c_sb[:, :, 32:40])
                # cjd[j][p, s, 0:2] = coef_j[p, s] twice (stride-1 pair so the
                # gate ops' broadcast operand keeps the DVE 2x perf mode)
                cjd = [coefp.tile([128, NQ * 8, 2], BF16, tag=f"c{j}",
                                  name=f"cjd{j}") for j in range(4)]
                for j in range(4):
                    dst = cjd[j][:].rearrange("p (q m) t -> p q m t", m=8)
                    cj_s = (c_sb[:, :, j * 8:(j + 1) * 8].unsqueeze(-1)
                            .to_broadcast([128, NQ, 8, 2]))
                    ri_s = rinv[:].unsqueeze(-1).to_broadcast([128, NQ, 8, 2])
                    nc.vector.tensor_mul(dst, cj_s, ri_s)

                # ---- gather + gate over chunks ----
                if l == 0:
                    src_ap = xT[:, :32]
                else:
                    src_ap = h_dram[l - 1][H_BASE:W, :32]

                for s0, ns in _chunks():
                    a_t = gath.tile([128, CHUNK_SLOTS, 32], BF16, tag="a")
                    b_t = gath.tile([128, CHUNK_SLOTS, 32], BF16, tag="b")
                    col = s0 * 8  # idx cols consumed so far (128/16 per slot)
                    slot = 0
                    for n in _gathers(ns):
                        ncols = n // 16
                        nslots_g = n // 128
                        dma_gather_small(
                            nc.gpsimd, a_t[:, slot:slot + nslots_g, :], src_ap,
                            ia_sb[:, col:col + ncols], n, 32, E)
                        dma_gather_small(
                            nc.gpsimd, b_t[:, slot:slot + nslots_g, :], src_ap,
                            ib_sb[:, col:col + ncols], n, 32, E)
                        col += ncols
                        slot += nslots_g

                    # 4-dim views with stride-1 inner pairs keep DVE 2x mode
                    av = a_t[:, :ns, :].rearrange("p c (g t) -> p c g t", t=2)
                    bv = b_t[:, :ns, :].rearrange("p c (g t) -> p c g t", t=2)

                    def cbc(j):
                        return (cjd[j][:, s0:s0 + ns, :].unsqueeze(2)
                                .to_broadcast([128, ns, 16, 2]))

                    # gate = (c1 + c3*b)*a + (c0 + c2*b): 6 DVE ops
                    m1 = temps.tile([128, CHUNK_SLOTS, 32], BF16, tag="m1")
                    m2 = temps.tile([128, CHUNK_SLOTS, 32], BF16, tag="m2")
                    m1v = m1[:, :ns, :].rearrange("p c (g t) -> p c g t", t=2)
                    m2v = m2[:, :ns, :].rearrange("p c (g t) -> p c g t", t=2)
                    nc.vector.tensor_mul(m1v, bv, cbc(3))
                    nc.vector.tensor_add(m1v, m1v, cbc(1))
                    nc.vector.tensor_mul(m1v, m1v, av)
                    nc.vector.tensor_mul(m2v, bv, cbc(2))
                    nc.vector.tensor_add(m2v, m2v, cbc(0))
                    nc.vector.tensor_add(m1v, m1v, m2v)

                    if l < 2:
                        # write rows (s0+c)*128+p of h_dram[l] (64B @ 256B stride)
                        hap = h_dram[l].ap()
                        dst = hap[s0 * 128: s0 * 128 + ns * 128, :32]
                        dst = dst.rearrange("(c p) e -> p c e", p=128)
                        nc.sync.dma_start(dst, m1[:, :ns, :])
                    else:
                        for c in range(ns):
                            nc.tensor.matmul(
                                psum_out[:],
                                lhsT=g10_sb[:, s0 + c, :],
                                rhs=m1[:, c, :],
                                start=(mm_i == 0),
                                stop=(mm_i == n_mm - 1),
                            )
                            mm_i += 1

            out_sb = persist.tile([K, BC], F32, tag="outsb")
            nc.scalar.mul(out_sb[:], psum_out[:], 1.0 / TAU)
            nc.sync.dma_start(out_dram[:], out_sb[:])

    nc.compile()
    _NC_CACHE["nc"] = nc
    return nc


def _wrap(idx):
    """Flat idx list [n] -> [128, n/16] int16 wrapped per 16 partitions,
    replicated to the 8 gpsimd cores."""
    n = idx.shape[0]
    arr = np.empty((128, n // 16), dtype=np.int16)
    blk = idx.reshape(n // 16, 16).T.astype(np.int16)
    for g in range(8):
        arr[g * 16:(g + 1) * 16, :] = blk
    return arr


def _fix_trailing(idx_a, idx_b):
    """Ensure the last idx of every GPN-sublist is >= 0 for both lists
    (SWDGE trims trailing negatives). Returns permuted lists + perm."""
    perm = np.arange(W)
    a = idx_a.copy()
    b = idx_b.copy()
    pos = 0
    for s0, ns in _chunks():
        for n in _gathers(ns):
            last = pos + n - 1
            if a[last] < 0 or b[last] < 0:
                ok = np.nonzero((a[pos:last] >= 0) & (b[pos:last] >= 0))[0]
                j = pos + int(ok[-1])
                for arr in (a, b, perm):
                    arr[last], arr[j] = arr[j], arr[last]
            pos += n
    return a, b, perm


def _fold(x):
    """[W, ...] -> [128, NSLOT, ...] with row n=(c*128+p) at [p, c]."""
    return np.ascontiguousarray(
        x.reshape(NSLOT, 128, *x.shape[1:]).transpose(1, 0, *range(2, x.ndim + 1)))


def kernel(x, w1, w2, w3, idx_a1, idx_b1, idx_a2, idx_b2, idx_a3, idx_b3):
    x = np.asarray(x, dtype=np.float32)
    ws = [np.asarray(w, dtype=np.float32) for w in (w1, w2, w3)]
    ias = [np.asarray(i).astype(np.int64) for i in (idx_a1, idx_a2, idx_a3)]
    ibs = [np.asarray(i).astype(np.int64) for i in (idx_b1, idx_b2, idx_b3)]

    nc = build_nc()

    # ---- host-side index translation / layout prep (shared across cores) ----
    # layer 0: sources are x columns (0..1023), no rebase needed
    a0, b0, perm0 = ias[0].copy(), ibs[0].copy(), np.arange(W)
    perms = [perm0]
    lists = [(a0, b0)]
    for l in (1, 2):
        inv_prev = np.empty(W, dtype=np.int64)
        inv_prev[perms[l - 1]] = np.arange(W)
        ra = inv_prev[ias[l]] - H_BASE
        rb = inv_prev[ibs[l]] - H_BASE
        ra2, rb2, perm = _fix_trailing(ra, rb)
        perms.append(perm)
        lists.append((ra2, rb2))

    NQ = (NSLOT + 7) // 8
    shared = {}
    for l in range(3):
        a, b = lists[l]
        shared[f"ia{l}"] = _wrap(a)
        shared[f"ib{l}"] = _wrap(b)
        # wf2[k*8+m, q, K] = w_perm[(8q+m)*128 + K, k]
        wp = ws[l][perms[l]]                      # [W, 16]
        wf2 = np.zeros((128, NQ, 128), dtype=np.float32)
        for m in range(8):
            s_ids = 8 * np.arange(NQ) + m
            valid = s_ids < NSLOT
            n = s_ids[valid][:, None] * 128 + np.arange(128)[None, :]
            vals = wp[n, :].transpose(2, 0, 1)    # [16, nq_v, 128]
            tmp = np.zeros((16, NQ, 128), dtype=np.float32)
            tmp[:, valid, :] = vals
            wf2[np.arange(16) * 8 + m] = tmp
        shared[f"wf{l}"] = wf2.astype(ml_dtypes.bfloat16)

    group = perms[2] // (W // K)          # group id of neuron at list pos j
    g10 = np.zeros((W, K), dtype=np.float32)
    g10[np.arange(W), group] = 1.0
    shared["g10"] = _fold(g10).astype(ml_dtypes.bfloat16)

    G5 = np.zeros((16, 5), dtype=np.float32)
    G5[:, :4] = GATE_COEF
    G5[:, 4] = 1.0
    ghat = np.zeros((128, 40), dtype=np.float32)
    for k in range(16):
        for m in range(8):
            ghat[k * 8 + m, np.arange(5) * 8 + m] = G5[k]
    shared["ghat"] = ghat.astype(ml_dtypes.bfloat16)

    in_maps = []
    for c in range(NCORES):
        xc = x[c * BC:(c + 1) * BC]               # [32, 1024]
        xt = np.zeros((IN_DIM, E), dtype=ml_dtypes.bfloat16)
        xt[:, :BC] = xc.T.astype(ml_dtypes.bfloat16)
        m = dict(shared)
        m["xT"] = xt
        in_maps.append(m)

    res = run_bass_kernel_spmd(nc, in_maps, core_ids=list(range(NCORES)))

    out = np.empty((BATCH, K), dtype=np.float32)
    for c in range(NCORES):
        out[c * BC:(c + 1) * BC] = res.results[c]["out"].T
    return out


# revision 21
# speedup vs baseline: 1.3045x; 1.0488x over previous
"""DiffLogic network TRN2 kernel: 3 logic layers [B=256, W=64000] + GroupSum.

Sharding: pure data-parallel over batch across 8 cores (B=32/core), no
inter-core communication. Per core: activations h stored in DRAM as
[64000, 128] bf16 rows (256B stride, first 32 cols real = 64B payload).
Gathers a=h[idx_a], b=h[idx_b] via SWDGE dma_gather with elem_size=32
(64B descriptors — 7ns floor instead of 22.8ns for 256B) and signed
int16 indices (base at row 32000). h writes go over HWDGE (SP engine)
to keep the Pool engine free for gather descriptor generation.
Gate = (c1+c3*b)*a + (c0+c2*b) computed on DVE (6 tensor-tensor ops,
kept in the 2x perf mode by storing coefficients as stride-1 pairs);
coefs = softmax(w)@G via ACT exp + PE matmuls contracting the 16-gate
axis on the partition dim. GroupSum via PE one-hot matmul.
"""
import numpy as np
import ml_dtypes

import concourse.bass as bass
import concourse.tile as tile
import concourse.bacc as bacc
import concourse.mybir as mybir
from concourse import ap_utils
from concourse.bass_utils import run_bass_kernel_spmd
from concourse.library_config import mlp

W = 64000
BATCH = 256
NCORES = 8
BC = BATCH // NCORES        # 32 batch rows per core
IN_DIM = 1024
K = 10
TAU = 30.0
NSLOT = W // 128            # 500
E = 128                     # elements per h row (256B stride); [:32] real
CHUNK_SLOTS = 64            # neurons per chunk = 64*128 = 8192
GPN = 1024                  # idxs per dma_gather instruction (Q7 scratch cap)
H_BASE = 32000              # gather base row (signed int16 rebase)

GATE_COEF = np.array([
    [0., 0., 0., 0.], [0., 0., 0., 1.], [0., 1., 0., -1.], [0., 1., 0., 0.],
    [0., 0., 1., -1.], [0., 0., 1., 0.], [0., 1., 1., -2.], [0., 1., 1., -1.],
    [1., -1., -1., 1.], [1., -1., -1., 2.], [1., 0., -1., 0.], [1., 0., -1., 1.],
    [1., -1., 0., 0.], [1., -1., 0., 1.], [1., 0., 0., -1.], [1., 0., 0., 0.],
], dtype=np.float32)  # [16, 4]

BF16 = mybir.dt.bfloat16
F32 = mybir.dt.float32
I16 = mybir.dt.int16
IDX_COLS = W // 16  # wrapped idx tensor cols per list

_NC_CACHE = {}


def dma_gather_small(gp, out_ap, in_ap, idxs_ap, num_idxs, elem_size, elem_step):
    """dma_gather without the 256B elem-size assert (non-transpose, DRAM src).
    The 256B restriction only applies to transpose mode in the ucode; the
    source row stride (elem_step) must still be a multiple of 256B."""
    assert idxs_ap.dtype == mybir.dt.int16
    assert in_ap.dtype == out_ap.dtype
    assert ap_utils.ap_is_contiguous(out_ap.ap[1:])
    assert ap_utils.ap_is_contiguous(idxs_ap.ap[1:])
    assert in_ap.ap[-1][1] == out_ap.ap[-1][1] == elem_size
    assert in_ap.ap[0][0] == elem_step
    stride_bytes = elem_step * mybir.dt.size(in_ap.dtype)
    stride_256 = stride_bytes // 256
    assert stride_256 * 256 == stride_bytes and stride_256 < 256
    _in_ap = gp.lower_ap_dma(in_ap, for_custom_bir_dma=True)
    _idxs_ap = gp.lower_ap(idxs_ap)
    _out_ap = gp.lower_ap(out_ap)
    return gp.add_instruction(
        mybir.InstDMAGatherAnt(
            name=gp.bass.get_next_instruction_name(),
            ins=[*_in_ap, _idxs_ap, gp.lower_val_access(gp.to_reg(num_idxs))],
            outs=[_out_ap],
            transpose=False,
            num_idxs=num_idxs,
            elem_size=elem_size,
            stride_bytes_256=stride_256,
            gen_mode=0,
            single_packet=True,
            queue_num=0,
            sbuf_tokens_per_rank=0,
            sbuf_free_dim_per_rank=0,
            sbuf_free_dim_pad_per_rank=0,
            sbuf_byte_offset=0,
        ))


def _chunks():
    """Yield (slot0, nslots) chunks over the 500 slots."""
    s = 0
    while s < NSLOT:
        n = min(CHUNK_SLOTS, NSLOT - s)
        yield s, n
        s += n


def _gathers(nslots):
    """Split a chunk of nslots*128 idxs into per-instruction counts."""
    n = nslots * 128
    out = []
    while n > 0:
        g = min(GPN, n)
        out.append(g)
        n -= g
    return out


def build_nc(bounds=None):
    if "nc" in _NC_CACHE:
        return _NC_CACHE["nc"]
    nc = bacc.Bacc("TRN2", target_bir_lowering=False, debug=False,
                   enable_asserts=False, num_devices=NCORES)

    NQ = (NSLOT + 7) // 8  # 63 PE-coef matmuls (8 slots each, 4 pad)

    xT = nc.dram_tensor("xT", [IN_DIM, E], BF16, kind="ExternalInput")
    # wf2[(k,m), q, K] = w[neuron(p=K, s=8q+m), k]  (PE-coef layout)
    wf = [nc.dram_tensor(f"wf{l}", [128, NQ, 128], BF16, kind="ExternalInput")
          for l in range(3)]
    ia = [nc.dram_tensor(f"ia{l}", [128, IDX_COLS], I16, kind="ExternalInput")
          for l in range(3)]
    ib = [nc.dram_tensor(f"ib{l}", [128, IDX_COLS], I16, kind="ExternalInput")
          for l in range(3)]
    g10 = nc.dram_tensor("g10", [128, NSLOT, K], BF16, kind="ExternalInput")
    # ghat[(k,m), 5j+mm] = G[k, j] * (m == mm); j=4 row is ones (softmax denom)
    ghat = nc.dram_tensor("ghat", [128, 40], BF16, kind="ExternalInput")
    h_dram = [nc.dram_tensor(f"h{l}", [W, E], BF16, kind="Internal")
              for l in range(2)]
    out_dram = nc.dram_tensor("out", [K, BC], F32, kind="ExternalOutput")

    with tile.TileContext(nc) as tc:
        with (
            tc.tile_pool(name="persist", bufs=1) as persist,
            tc.tile_pool(name="idxp", bufs=2) as idxp,
            tc.tile_pool(name="coef", bufs=2) as coefp,
            tc.tile_pool(name="gath", bufs=2) as gath,
            tc.tile_pool(name="temps", bufs=2) as temps,
            tc.tile_pool(name="psum", bufs=1, space="PSUM") as psump,
        ):
            nc.gpsimd.load_library(mlp)

            # persistent loads
            ghat_sb = persist.tile([128, 40], BF16, tag="ghat")
            nc.sync.dma_start(ghat_sb[:], ghat[:])
            g10_sb = persist.tile([128, NSLOT, K], BF16, tag="g10")
            nc.sync.dma_start(g10_sb[:], g10[:])

            psum_out = psump.tile([K, BC], F32, tag="acc")
            n_mm = NSLOT  # total groupsum matmuls
            mm_i = 0

            for l in range(3):
                ia_sb = idxp.tile([128, IDX_COLS], I16, tag="ia", name="ia_sb")
                ib_sb = idxp.tile([128, IDX_COLS], I16, tag="ib", name="ib_sb")
                nc.sync.dma_start(ia_sb[:], ia[l][:])
                nc.sync.dma_start(ib_sb[:], ib[l][:])

                # ---- coefficient prep: coef = softmax(w) @ GATE_COEF via PE ----
                # craw[j, n] for neurons n=(K, s=8q+m): matmul q contracts over
                # the (gate k, m) partition dim: out[K, j*8+mm] = sum_p
                # e_fold[p, K] * ghat[p, j*8+mm].
                wf_sb = coefp.tile([128, NQ, 128], BF16, tag="wf", name="wf_sb")
                nc.sync.dma_start(wf_sb[:], wf[l][:])
                e_sb = coefp.tile([128, NQ, 128], BF16, tag="e", name="e_sb")
                nc.scalar.activation(e_sb[:], wf_sb[:],
                                     mybir.ActivationFunctionType.Exp)
                c_sb = coefp.tile([128, NQ, 40], BF16, tag="csb", name="c_sb")
                QG = 12  # matmuls per PSUM bank group
                q0 = 0
                while q0 < NQ:
                    nq = min(QG, NQ - q0)
                    cps = psump.tile([128, QG, 40], F32, tag="cps", name="cps")
                    for qi in range(nq):
                        nc.tensor.matmul(cps[:, qi, :],
                                         lhsT=e_sb[:, q0 + qi, :],
                                         rhs=ghat_sb[:],
                                         start=True, stop=True)
                    nc.scalar.mul(c_sb[:, q0:q0 + nq, :], cps[:, :nq, :], 1.0)
                    q0 += nq
                rinv = coefp.tile([128, NQ, 8], F32, tag="rinv", name="rinv")
                nc.vector.reciprocal(out=rinv[:], in_=c_sb[:, :, 32:40])
                # cjd[j][p, s, 0:2] = coef_j[p, s] twice (stride-1 pair so the
                # gate ops' broadcast operand keeps the DVE 2x perf mode)
                cjd = [coefp.tile([128, NQ * 8, 2], BF16, tag=f"c{j}",
                                  name=f"cjd{j}") for j in range(4)]
                for j in range(4):
                    dst = cjd[j][:].rearrange("p (q m) t -> p q m t", m=8)
                    cj_s = (c_sb[:, :, j * 8:(j + 1) * 8].unsqueeze(-1)
                            .to_broadcast([128, NQ, 8, 2]))
                    ri_s = rinv[:].unsqueeze(-1).to_broadcast([128, NQ, 8, 2])
                    nc.vector.tensor_mul(dst, cj_s, ri_s)

                # ---- gather + gate over chunks ----
                def srcs(gi):
                    """Per-instruction source windows: base from the per-
                    instruction int16 rebase, upper bound = max row touched
                    (consumers are sorted by source row, so early gathers
                    depend only on early h writes and can overlap the
                    previous layer's tail)."""
                    if l == 0:
                        return xT[:, :32], xT[:, :32]
                    assert bounds is not None, "layers 2-3 need rebase info"
                    ba, bda, bb, bdb = bounds[l - 1]
                    return (h_dram[l - 1][ba[gi]:bda[gi], :32],
                            h_dram[l - 1][bb[gi]:bdb[gi], :32])

                gi = 0
                for s0, ns in _chunks():
                    a_t = gath.tile([128, CHUNK_SLOTS, 32], BF16, tag="a")
                    b_t = gath.tile([128, CHUNK_SLOTS, 32], BF16, tag="b")
                    col = s0 * 8  # idx cols consumed so far (128/16 per slot)
                    slot = 0
                    for n in _gathers(ns):
                        ncols = n // 16
                        nslots_g = n // 128
                        src_a, src_b = srcs(gi)
                        gi += 1
                        dma_gather_small(
                            nc.gpsimd, a_t[:, slot:slot + nslots_g, :], src_a,
                            ia_sb[:, col:col + ncols], n, 32, E)
                        dma_gather_small(
                            nc.gpsimd, b_t[:, slot:slot + nslots_g, :], src_b,
                            ib_sb[:, col:col + ncols], n, 32, E)
                        col += ncols
                        slot += nslots_g

                    # 4-dim views with stride-1 inner pairs keep DVE 2x mode
                    av = a_t[:, :ns, :].rearrange("p c (g t) -> p c g t", t=2)
                    bv = b_t[:, :ns, :].rearrange("p c (g t) -> p c g t", t=2)

                    def cbc(j):
                        return (cjd[j][:, s0:s0 + ns, :].unsqueeze(2)
                                .to_broadcast([128, ns, 16, 2]))

                    # gate = (c1 + c3*b)*a + (c0 + c2*b): 6 DVE ops
                    m1 = temps.tile([128, CHUNK_SLOTS, 32], BF16, tag="m1")
                    m2 = temps.tile([128, CHUNK_SLOTS, 32], BF16, tag="m2")
                    m1v = m1[:, :ns, :].rearrange("p c (g t) -> p c g t", t=2)
                    m2v = m2[:, :ns, :].rearrange("p c (g t) -> p c g t", t=2)
                    nc.vector.tensor_mul(m1v, bv, cbc(3))
                    nc.vector.tensor_add(m1v, m1v, cbc(1))
                    nc.vector.tensor_mul(m1v, m1v, av)
                    nc.vector.tensor_mul(m2v, bv, cbc(2))
                    nc.vector.tensor_add(m2v, m2v, cbc(0))
                    nc.vector.tensor_add(m1v, m1v, m2v)

                    if l < 2:
                        # write rows (s0+c)*128+p of h_dram[l] (64B @ 256B stride)
                        hap = h_dram[l].ap()
                        dst = hap[s0 * 128: s0 * 128 + ns * 128, :32]
                        dst = dst.rearrange("(c p) e -> p c e", p=128)
                        nc.sync.dma_start(dst, m1[:, :ns, :])
                    else:
                        for c in range(ns):
                            nc.tensor.matmul(
                                psum_out[:],
                                lhsT=g10_sb[:, s0 + c, :],
                                rhs=m1[:, c, :],
                                start=(mm_i == 0),
                                stop=(mm_i == n_mm - 1),
                            )
                            mm_i += 1

            out_sb = persist.tile([K, BC], F32, tag="outsb")
            nc.scalar.mul(out_sb[:], psum_out[:], 1.0 / TAU)
            nc.sync.dma_start(out_dram[:], out_sb[:])

    nc.compile()
    _NC_CACHE["nc"] = nc
    return nc


def _wrap(idx):
    """Flat idx list [n] -> [128, n/16] int16 wrapped per 16 partitions,
    replicated to the 8 gpsimd cores."""
    n = idx.shape[0]
    arr = np.empty((128, n // 16), dtype=np.int16)
    blk = idx.reshape(n // 16, 16).T.astype(np.int16)
    for g in range(8):
        arr[g * 16:(g + 1) * 16, :] = blk
    return arr


def _fix_trailing(idx_a, idx_b, perm=None):
    """Ensure the last idx of every GPN-sublist is >= 0 for both lists
    (SWDGE trims trailing negatives). Returns permuted lists + perm."""
    perm = np.arange(W) if perm is None else perm.copy()
    a = idx_a.copy()
    b = idx_b.copy()
    pos = 0
    for s0, ns in _chunks():
        for n in _gathers(ns):
            last = pos + n - 1
            if a[last] < 0 or b[last] < 0:
                ok = np.nonzero((a[pos:last] >= 0) & (b[pos:last] >= 0))[0]
                j = pos + int(ok[-1])
                for arr in (a, b, perm):
                    arr[last], arr[j] = arr[j], arr[last]
            pos += n
    return a, b, perm


def _fold(x):
    """[W, ...] -> [128, NSLOT, ...] with row n=(c*128+p) at [p, c]."""
    return np.ascontiguousarray(
        x.reshape(NSLOT, 128, *x.shape[1:]).transpose(1, 0, *range(2, x.ndim + 1)))


def kernel(x, w1, w2, w3, idx_a1, idx_b1, idx_a2, idx_b2, idx_a3, idx_b3):
    x = np.asarray(x, dtype=np.float32)
    ws = [np.asarray(w, dtype=np.float32) for w in (w1, w2, w3)]
    ias = [np.asarray(i).astype(np.int64) for i in (idx_a1, idx_a2, idx_a3)]
    ibs = [np.asarray(i).astype(np.int64) for i in (idx_b1, idx_b2, idx_b3)]

    # ---- host-side index translation / layout prep (shared across cores) ----
    # layer 0: sources are x columns (0..1023), no rebase needed
    a0, b0, perm0 = ias[0].copy(), ibs[0].copy(), np.arange(W)
    perms = [perm0]
    lists = [(a0, b0)]
    bounds = []
    for l in (1, 2):
        inv_prev = np.empty(W, dtype=np.int64)
        inv_prev[perms[l - 1]] = np.arange(W)
        ra = inv_prev[ias[l]]
        rb = inv_prev[ibs[l]]
        # sort consumers by max source row: early gather instructions then
        # only touch early h rows, so (with per-instruction src-AP windows)
        # they can start before the previous layer finishes writing h
        order = np.argsort(np.maximum(ra, rb), kind="stable")
        ra2, rb2 = ra[order].copy(), rb[order].copy()
        # per-instruction int16 rebase: base = max(0, hi-32767) keeps every
        # idx in range; sorted sublists have narrow spans so most have no
        # negative idxs at all
        binfo = ([], [], [], [])  # base_a, bound_a, base_b, bound_b
        pos = 0
        for s0, ns in _chunks():
            for n in _gathers(ns):
                sl = slice(pos, pos + n)
                for arr, k in ((ra2, 0), (rb2, 2)):
                    hi = int(arr[sl].max())
                    base = max(0, hi - 32767)
                    arr[sl] -= base
                    binfo[k].append(base)
                    binfo[k + 1].append(hi + 1)
                pos += n
        ra3, rb3, perm = _fix_trailing(ra2, rb2, perm=order)
        perms.append(perm)
        lists.append((ra3, rb3))
        bounds.append(binfo)

    nc = build_nc(bounds)

    NQ = (NSLOT + 7) // 8
    shared = {}
    for l in range(3):
        a, b = lists[l]
        shared[f"ia{l}"] = _wrap(a)
        shared[f"ib{l}"] = _wrap(b)
        # wf2[k*8+m, q, K] = w_perm[(8q+m)*128 + K, k]
        wp = ws[l][perms[l]]                      # [W, 16]
        wf2 = np.zeros((128, NQ, 128), dtype=np.float32)
        for m in range(8):
            s_ids = 8 * np.arange(NQ) + m
            valid = s_ids < NSLOT
            n = s_ids[valid][:, None] * 128 + np.arange(128)[None, :]
            vals = wp[n, :].transpose(2, 0, 1)    # [16, nq_v, 128]
            tmp = np.zeros((16, NQ, 128), dtype=np.float32)
            tmp[:, valid, :] = vals
            wf2[np.arange(16) * 8 + m] = tmp
        shared[f"wf{l}"] = wf2.astype(ml_dtypes.bfloat16)

    group = perms[2] // (W // K)          # group id of neuron at list pos j
    g10 = np.zeros((W, K), dtype=np.float32)
    g10[np.arange(W), group] = 1.0
    shared["g10"] = _fold(g10).astype(ml_dtypes.bfloat16)

    G5 = np.zeros((16, 5), dtype=np.float32)
    G5[:, :4] = GATE_COEF
    G5[:, 4] = 1.0
    ghat = np.zeros((128, 40), dtype=np.float32)
    for k in range(16):
        for m in range(8):
            ghat[k * 8 + m, np.arange(5) * 8 + m] = G5[k]
    shared["ghat"] = ghat.astype(ml_dtypes.bfloat16)

    in_maps = []
    for c in range(NCORES):
        xc = x[c * BC:(c + 1) * BC]               # [32, 1024]
        xt = np.zeros((IN_DIM, E), dtype=ml_dtypes.bfloat16)
        xt[:, :BC] = xc.T.astype(ml_dtypes.bfloat16)
        m = dict(shared)
        m["xT"] = xt
        in_maps.append(m)

    res = run_bass_kernel_spmd(nc, in_maps, core_ids=list(range(NCORES)))

    out = np.empty((BATCH, K), dtype=np.float32)
    for c in range(NCORES):
        out[c * BC:(c + 1) * BC] = res.results[c]["out"].T
    return out
